# revision 7
# baseline (speedup 1.0000x reference)
"""MoE top-k routing kernel for Trainium2 (nn_MixedOp: top-2 of 8 Dense(1024->1024)+relu, summed).

Strategy:
  - Host: top-k selection over the 8 logits (tiny), slice the k selected expert
    weights/biases, transpose x so the contraction dim (D) is the SBUF
    partition dim (cast to the internal compute dtype).
  - Device: data-parallel shard of the 8192-token batch across 8 NeuronCores
    (1024 tokens/core), no collectives. Each core computes
        outT[:, t] = sum_e relu(W_e^T @ xT[:, t] + b_e)
    with PE matmuls (fp32 PSUM accumulate), relu+bias fused on the scalar
    engine, expert-sum on the vector engine. Expert-outer loop so expert e+1
    weights stream from HBM while expert e computes; the first expert runs
    dk-major over 4 concurrent PSUM groups so the PE never waits on the HBM
    fill; garbage warmup matmuls trip the PE clock gate to 2.4 GHz during the
    fill. x rides sync's HWDGE queue, W rides scalar's, in consumption order
    (each dma_start costs ~0.65us of sequencer issue time, and completion
    fires per whole transfer, so queue order = arrival order).
  - Host: transpose per-core outputs back and concatenate.

Measured (8 cores, bf16): 72.8-75us HW exec (best 72,842 ns), max-rel-err
~2.3e-3, resid_var ~4e-6 vs the fp32 reference. PE roofline ~55us; the rest
is the measured framework floor (~7us BSP preamble, ~4us HBM gating latency,
~6us exit protocol) — all verified invariant to kernel structure.
"""

import os
import sys
from contextlib import ExitStack

if "/opt/trn_rl_repo" not in sys.path:
    sys.path.insert(0, "/opt/trn_rl_repo")

import numpy as np
import ml_dtypes

import concourse.tile as tile
import concourse.bacc as bacc
import concourse.mybir as mybir
from concourse.bass_utils import run_bass_kernel_spmd

# bass_utils imports antenv.axon_hooks when tracing is requested (e.g. via a
# BASS_TRACE env var); the module is absent on some agent images — stub it so
# that path degrades to an untraced run instead of an ImportError.
try:
    import antenv.axon_hooks  # noqa: F401
except ImportError:
    import types as _types
    _m = _types.ModuleType("antenv.axon_hooks")
    _m.get_axon_ntff_profile_hook = lambda: None
    _m.set_axon_ntff_profile_hook = lambda h: None
    sys.modules["antenv.axon_hooks"] = _m

NCORES = 8
B = 8192
D = 1024
TPC = B // NCORES      # tokens per core
P = 128                # SBUF partitions
NT = 512               # matmul moving free-dim tile (one fp32 PSUM bank)
DK = D // P            # contraction tiles (8)
EM = D // P            # output-dim tiles (8)
TN = TPC // NT         # token tiles per core (2)

# internal compute dtype: "bf16" | "f32r" (fp32 data, full-rate reduced-precision
# PE mode) | "f32" (native fp32, 4x slower PE)
_DTYPE = os.environ.get("MOE_DTYPE", "bf16")
# of garbage matmuls appended after the real stream: keeps the PE activity
# monitor at 8/8 clock through the framework's exit protocol (otherwise the
# ~255-semaphore reset sweep runs at 4/8 clock and takes ~2x longer)
_TAIL_WARM = int(os.environ.get("MOE_TAIL_WARM", "100"))
# store outputs as bf16 (half the store traffic; adds <=2^-9 relative error)
_BF16_OUT = os.environ.get("MOE_BF16_OUT", "1") == "1"
# split the last em-block of the last expert into 256-token chunks so the
# final relu+add+store chain after the very last matmul is short
_FINE_TAIL = os.environ.get("MOE_FINE_TAIL", "1") == "1"

_nc_cache = {}


def _mdt(dtype: str):
    return {
        "bf16": mybir.dt.bfloat16,
        "f32r": mybir.dt.float32r,
        "f32": mybir.dt.float32,
    }[dtype]


def _npdt(dtype: str):
    return ml_dtypes.bfloat16 if dtype == "bf16" else np.float32


def _build(k: int, dtype: str):
    mdt = _mdt(dtype)
    f32 = mybir.dt.float32
    odt = mybir.dt.bfloat16 if _BF16_OUT else f32
    nc = bacc.Bacc("TRN2", debug=False, target_bir_lowering=False, num_devices=NCORES)
    xT_ap = nc.dram_tensor("xT", [D, TPC], mdt, kind="ExternalInput").ap()
    w_ap = nc.dram_tensor("w", [k, D, D], mdt, kind="ExternalInput").ap()
    bT_ap = nc.dram_tensor("bT", [P, k * EM], f32, kind="ExternalInput").ap()
    outT_ap = nc.dram_tensor("outT", [D, TPC], odt, kind="ExternalOutput").ap()

    with tile.TileContext(nc) as tc:
        with ExitStack() as ctx:
            xpool = ctx.enter_context(tc.tile_pool(name="x", bufs=1))
            wpool = ctx.enter_context(tc.tile_pool(name="w", bufs=1))
            bpool = ctx.enter_context(tc.tile_pool(name="b", bufs=1))
            pspool = ctx.enter_context(tc.tile_pool(name="ps", bufs=8, space="PSUM"))
            rpool = ctx.enter_context(tc.tile_pool(name="r", bufs=4))
            opool = ctx.enter_context(tc.tile_pool(name="o", bufs=4))
            apool = ctx.enter_context(tc.tile_pool(name="acc", bufs=1))

            # Queue discipline: HWDGE queues are per-engine FIFOs and a DMA's
            # completion semaphore fires only when the whole transfer is done,
            # so what shares a queue (and when) controls when the PE's gating
            # tiles land. x (+bias, +outputs later) ride sync's queue; W strips
            # ride scalar's queue in exact consumption order (expert 0 first).
            # wide tiles with per-strip DMAs into slices: slice-level dep
            # tracking keeps per-strip gating while using 1 pool slot each
            x_big = xpool.tile([P, DK * TPC], mdt, tag="xbig")
            xs = []
            for dk in range(DK):
                t = x_big[:, dk * TPC:(dk + 1) * TPC]
                nc.sync.dma_start(out=t, in_=xT_ap[dk * P:(dk + 1) * P, :])
                xs.append(t)

            # bias is tiny and first needed ~20us in; keep it off the head of
            # the x queue
            bias = bpool.tile([P, k * EM], f32, tag="bias")
            nc.sync.dma_start(out=bias[:], in_=bT_ap[:])

            ws = {}
            for e in range(k):
                w_big = wpool.tile([P, DK * D], mdt, name=f"w_big_{e}",
                                   tag=f"wbig{e}")
                for dk in range(DK):
                    t = w_big[:, dk * D:(dk + 1) * D]
                    nc.scalar.dma_start(out=t, in_=w_ap[e, dk * P:(dk + 1) * P, :])
                    ws[e, dk] = t

            # ~4us of garbage matmuls while the HBM fill runs: trips the PE
            # HAM activity monitor to 8/8 (2.4 GHz) so the real stream starts
            # warm instead of paying ~2x on its first ~3.4us.
            wmt = bpool.tile([P, 64], mybir.dt.bfloat16, tag="warm")
            nc.vector.memset(wmt[:], 0)
            wps = pspool.tile([P, 64], f32, name="ps_warm", tag="ps")
            for i in range(90):
                nc.tensor.matmul(wps[0:64, :], wmt[:], wmt[:], start=True, stop=True)

            # persistent accumulator: one wide tile, sliced per (em,tn).
            # Slice-level deps proved structurally neutral vs 16 separate
            # tiles, and 15 fewer pool slots shortens the exit-protocol
            # semaphore sweep.
            acc_big = apool.tile([P, EM * TN * NT], f32, tag="accbig")
            accs = {}

            def epilogue(e, em, ps, tn_list=None, cols=None):
                bias_col = bias[:, e * EM + em: e * EM + em + 1]
                lo, hi = cols if cols is not None else (0, NT)
                for tn in (tn_list if tn_list is not None else range(TN)):
                    if e == 0:
                        i = em * TN + tn
                        acc = acc_big[:, i * NT:(i + 1) * NT]
                        accs[em, tn] = acc
                        if k == 1:
                            o = opool.tile([P, hi - lo], odt,
                                           name=f"o_{em}_{tn}_{lo}", tag="o")
                            nc.scalar.activation(
                                o[:], ps[tn][:, lo:hi],
                                mybir.ActivationFunctionType.Relu,
                                bias=bias_col)
                            nc.sync.dma_start(
                                out=outT_ap[em * P:(em + 1) * P,
                                            tn * NT + lo:tn * NT + hi],
                                in_=o[:])
                        else:
                            nc.scalar.activation(
                                acc[:, lo:hi], ps[tn][:, lo:hi],
                                mybir.ActivationFunctionType.Relu,
                                bias=bias_col)
                    elif e == k - 1:
                        acc = accs[em, tn]
                        r = rpool.tile([P, hi - lo], f32,
                                       name=f"r_{e}_{em}_{tn}_{lo}", tag="r")
                        nc.scalar.activation(
                            r[:], ps[tn][:, lo:hi],
                            mybir.ActivationFunctionType.Relu, bias=bias_col)
                        o = opool.tile([P, hi - lo], odt,
                                       name=f"o_{em}_{tn}_{lo}", tag="o")
                        nc.vector.tensor_add(o[:], acc[:, lo:hi], r[:])
                        nc.sync.dma_start(
                            out=outT_ap[em * P:(em + 1) * P,
                                        tn * NT + lo:tn * NT + hi],
                            in_=o[:])
                    else:
                        acc = accs[em, tn]
                        r = rpool.tile([P, hi - lo], f32,
                                       name=f"r_{e}_{em}_{tn}_{lo}", tag="r")
                        nc.scalar.activation(
                            r[:], ps[tn][:, lo:hi],
                            mybir.ActivationFunctionType.Relu, bias=bias_col)
                        nc.vector.tensor_add(acc[:, lo:hi], acc[:, lo:hi],
                                             r[:])

            GW = 8 // TN  # em-groups per sweep (TN*GW psum banks in flight)
            for e in range(k):
                if e == 0:
                    # dk-major over GW concurrent groups: every arriving x/W
                    # strip immediately feeds TN*GW matmuls, so the PE never
                    # stalls on the HBM fill at kernel start.
                    for half in range(EM // GW):
                        groups = range(GW * half, GW * half + GW)
                        ps = {
                            g: [pspool.tile([P, NT], f32,
                                            name=f"ps_{e}_{g}_{tn}", tag="ps")
                                for tn in range(TN)]
                            for g in groups
                        }
                        for dk in range(DK):
                            for g in groups:
                                lhsT = ws[e, dk][:, g * P:(g + 1) * P]
                                for tn in range(TN):
                                    nc.tensor.matmul(
                                        ps[g][tn][:], lhsT,
                                        xs[dk][:, tn * NT:(tn + 1) * NT],
                                        start=(dk == 0), stop=(dk == DK - 1))
                        for g in groups:
                            epilogue(e, g, ps[g])
                else:
                    # data resident by now: plain em-major streaming
                    for em in range(EM):
                        ps = [
                            pspool.tile([P, NT], f32,
                                        name=f"ps_{e}_{em}_{tn}", tag="ps")
                            for tn in range(TN)
                        ]
                        if em == EM - 1 and e == k - 1 and _FINE_TAIL:
                            # last tile: 256-token chunks, each fully finished
                            # (matmuls + relu/add/store) before the next, so
                            # the post-last-matmul drain is one short chunk
                            CH = NT // 2
                            for tn in range(TN):
                                for c in range(NT // CH):
                                    for dk in range(DK):
                                        nc.tensor.matmul(
                                            ps[tn][:, c * CH:(c + 1) * CH],
                                            ws[e, dk][:, em * P:(em + 1) * P],
                                            xs[dk][:, tn * NT + c * CH:
                                                    tn * NT + (c + 1) * CH],
                                            start=(dk == 0),
                                            stop=(dk == DK - 1))
                                    epilogue(e, em, ps, tn_list=[tn],
                                             cols=(c * CH, (c + 1) * CH))
                            continue
                        if em == EM - 1:
                            # tail: finish tile tn=0 completely first so its
                            # relu/add/store chain overlaps tn=1's matmuls
                            for tn in range(TN):
                                for dk in range(DK):
                                    nc.tensor.matmul(
                                        ps[tn][:],
                                        ws[e, dk][:, em * P:(em + 1) * P],
                                        xs[dk][:, tn * NT:(tn + 1) * NT],
                                        start=(dk == 0), stop=(dk == DK - 1))
                        else:
                            for dk in range(DK):
                                lhsT = ws[e, dk][:, em * P:(em + 1) * P]
                                for tn in range(TN):
                                    nc.tensor.matmul(
                                        ps[tn][:], lhsT,
                                        xs[dk][:, tn * NT:(tn + 1) * NT],
                                        start=(dk == 0), stop=(dk == DK - 1))
                        epilogue(e, em, ps)

            # post-stream keep-alive: the exit protocol's ~255-semaphore
            # reset sweep runs at whatever clock the activity monitor last
            # settled on; idle-gating to 4/8 doubles its length. Burn tiny
            # garbage matmuls through the epilogue-drain window to hold 8/8.
            if _TAIL_WARM:
                gps = [pspool.tile([P, 64], f32, name=f"gps{j}", tag="ps")
                       for j in range(2)]
                for i in range(_TAIL_WARM):
                    nc.tensor.matmul(gps[i % 2][0:64, :], wmt[:], wmt[:],
                                     start=True, stop=True)

    nc.compile()
    return nc


def _get_nc(k: int, dtype: str):
    key = (k, dtype)
    if key not in _nc_cache:
        _nc_cache[key] = _build(k, dtype)
    return _nc_cache[key]


def _prep_in_maps(x, logits, Ws, bs, k, dtype):
    x = np.asarray(x, dtype=np.float32)
    logits = np.asarray(logits, dtype=np.float32)
    Ws = np.asarray(Ws, dtype=np.float32)
    bs = np.asarray(bs, dtype=np.float32)

    # top-k by logits, descending, ties -> lower index (matches jax.lax.top_k)
    ids = np.argsort(-logits, kind="stable")[:k]

    npdt = _npdt(dtype)
    Wd = np.ascontiguousarray(Ws[ids].astype(npdt))              # [k, D, D]
    bT = np.ascontiguousarray(
        bs[ids].reshape(k, EM, P).transpose(2, 0, 1).reshape(P, k * EM)
    ).astype(np.float32)                                         # [P, k*EM]
    xT = x.astype(npdt).T                                        # [D, B] view

    in_maps = []
    for c in range(NCORES):
        in_maps.append({
            "xT": np.ascontiguousarray(xT[:, c * TPC:(c + 1) * TPC]),
            "w": Wd,
            "bT": bT,
        })
    return in_maps


def _gather(results):
    out = np.empty((B, D), dtype=np.float32)
    for c in range(NCORES):
        out[c * TPC:(c + 1) * TPC, :] = \
            np.asarray(results[c]["outT"]).astype(np.float32).T
    return out


def kernel(x, logits, Ws, bs, num_on_samples):
    k = int(num_on_samples)
    in_maps = _prep_in_maps(x, logits, Ws, bs, k, _DTYPE)
    nc = _get_nc(k, _DTYPE)
    res = run_bass_kernel_spmd(nc, in_maps, list(range(NCORES)))
    return _gather(res.results)


def run_traced(x, logits, Ws, bs, num_on_samples, dtype=None, **spmd_kwargs):
    """Dev helper: same as kernel() but returns (output, BassKernelResults)."""
    k = int(num_on_samples)
    dtype = dtype or _DTYPE
    in_maps = _prep_in_maps(x, logits, Ws, bs, k, dtype)
    nc = _get_nc(k, dtype)
    res = run_bass_kernel_spmd(nc, in_maps, list(range(NCORES)), **spmd_kwargs)
    return _gather(res.results), res



# revision 11
# speedup vs baseline: 1.0058x; 1.0058x over previous
"""MoE top-k routing kernel for Trainium2 (nn_MixedOp: top-2 of 8 Dense(1024->1024)+relu, summed).

Strategy:
  - Host: top-k selection over the 8 logits (tiny), slice the k selected expert
    weights/biases, transpose x so the contraction dim (D) is the SBUF
    partition dim (cast to the internal compute dtype).
  - Device: data-parallel shard of the 8192-token batch across 8 NeuronCores
    (1024 tokens/core), no collectives. Each core computes
        outT[:, t] = sum_e relu(W_e^T @ xT[:, t] + b_e)
    with PE matmuls (fp32 PSUM accumulate), relu+bias fused on the scalar
    engine, expert-sum on the vector engine. Expert-outer loop so expert e+1
    weights stream from HBM while expert e computes; the first expert runs
    dk-major over 4 concurrent PSUM groups so the PE never waits on the HBM
    fill; garbage warmup matmuls trip the PE clock gate to 2.4 GHz during the
    fill. x rides sync's HWDGE queue, W rides scalar's, in consumption order
    (each dma_start costs ~0.65us of sequencer issue time, and completion
    fires per whole transfer, so queue order = arrival order).
  - Host: transpose per-core outputs back and concatenate.

Measured (8 cores, bf16): 72.8-75us HW exec (best 72,842 ns), max-rel-err
~2.3e-3, resid_var ~4e-6 vs the fp32 reference. PE roofline ~55us; the rest
is the measured framework floor (~7us BSP preamble, ~4us HBM gating latency,
~6us exit protocol) — all verified invariant to kernel structure.
"""

import os
import sys
from contextlib import ExitStack

if "/opt/trn_rl_repo" not in sys.path:
    sys.path.insert(0, "/opt/trn_rl_repo")

import numpy as np
import ml_dtypes

import concourse.tile as tile
import concourse.bacc as bacc
import concourse.mybir as mybir
from concourse.bass_utils import run_bass_kernel_spmd

# bass_utils imports antenv.axon_hooks when tracing is requested (e.g. via a
# BASS_TRACE env var); the module is absent on some agent images — stub it so
# that path degrades to an untraced run instead of an ImportError.
try:
    import antenv.axon_hooks  # noqa: F401
except ImportError:
    import types as _types
    _m = _types.ModuleType("antenv.axon_hooks")
    _m.get_axon_ntff_profile_hook = lambda: None
    _m.set_axon_ntff_profile_hook = lambda h: None
    sys.modules["antenv.axon_hooks"] = _m

NCORES = 8
B = 8192
D = 1024
TPC = B // NCORES      # tokens per core
P = 128                # SBUF partitions
NT = 512               # matmul moving free-dim tile (one fp32 PSUM bank)
DK = D // P            # contraction tiles (8)
EM = D // P            # output-dim tiles (8)
TN = TPC // NT         # token tiles per core (2)

# internal compute dtype: "bf16" | "f32r" (fp32 data, full-rate reduced-precision
# PE mode) | "f32" (native fp32, 4x slower PE)
_DTYPE = os.environ.get("MOE_DTYPE", "bf16")
# of garbage matmuls appended after the real stream. Measured: the exit
# protocol's semaphore sweep paces at ~115ns/reset regardless of the HAM
# clock state, so keeping the clock up through the exit buys nothing.
_TAIL_WARM = int(os.environ.get("MOE_TAIL_WARM", "0"))
# store outputs as bf16 (half the store traffic; adds <=2^-9 relative error)
_BF16_OUT = os.environ.get("MOE_BF16_OUT", "1") == "1"
# split the last em-block of the last expert into 256-token chunks so the
# final relu+add+store chain after the very last matmul is short
_FINE_TAIL = os.environ.get("MOE_FINE_TAIL", "1") == "1"

_nc_cache = {}


def _mdt(dtype: str):
    return {
        "bf16": mybir.dt.bfloat16,
        "f32r": mybir.dt.float32r,
        "f32": mybir.dt.float32,
    }[dtype]


def _npdt(dtype: str):
    return ml_dtypes.bfloat16 if dtype == "bf16" else np.float32


def _build(k: int, dtype: str):
    mdt = _mdt(dtype)
    f32 = mybir.dt.float32
    odt = mybir.dt.bfloat16 if _BF16_OUT else f32
    nc = bacc.Bacc("TRN2", debug=False, target_bir_lowering=False, num_devices=NCORES)
    xT_ap = nc.dram_tensor("xT", [D, TPC], mdt, kind="ExternalInput").ap()
    w_ap = nc.dram_tensor("w", [k, D, D], mdt, kind="ExternalInput").ap()
    bT_ap = nc.dram_tensor("bT", [P, k * EM], f32, kind="ExternalInput").ap()
    outT_ap = nc.dram_tensor("outT", [D, TPC], odt, kind="ExternalOutput").ap()

    with tile.TileContext(nc) as tc:
        with ExitStack() as ctx:
            xpool = ctx.enter_context(tc.tile_pool(name="x", bufs=1))
            wpool = ctx.enter_context(tc.tile_pool(name="w", bufs=1))
            bpool = ctx.enter_context(tc.tile_pool(name="b", bufs=1))
            pspool = ctx.enter_context(tc.tile_pool(name="ps", bufs=8, space="PSUM"))
            rpool = ctx.enter_context(tc.tile_pool(name="r", bufs=4))
            opool = ctx.enter_context(tc.tile_pool(name="o", bufs=4))
            apool = ctx.enter_context(tc.tile_pool(name="acc", bufs=1))

            # Queue discipline: HWDGE queues are per-engine FIFOs and a DMA's
            # completion semaphore fires only when the whole transfer is done,
            # so what shares a queue (and when) controls when the PE's gating
            # tiles land. x (+bias, +outputs later) ride sync's queue; W strips
            # ride scalar's queue in exact consumption order (expert 0 first).
            # wide tiles with per-strip DMAs into slices: slice-level dep
            # tracking keeps per-strip gating while using 1 pool slot each
            x_big = xpool.tile([P, DK * TPC], mdt, tag="xbig")
            xs = []
            for dk in range(DK):
                t = x_big[:, dk * TPC:(dk + 1) * TPC]
                nc.sync.dma_start(out=t, in_=xT_ap[dk * P:(dk + 1) * P, :])
                xs.append(t)

            # bias is tiny and first needed ~20us in; keep it off the head of
            # the x queue
            bias = bpool.tile([P, k * EM], f32, tag="bias")
            nc.sync.dma_start(out=bias[:], in_=bT_ap[:])

            ws = {}
            for e in range(k):
                w_big = wpool.tile([P, DK * D], mdt, name=f"w_big_{e}",
                                   tag=f"wbig{e}")
                for dk in range(DK):
                    t = w_big[:, dk * D:(dk + 1) * D]
                    nc.scalar.dma_start(out=t, in_=w_ap[e, dk * P:(dk + 1) * P, :])
                    ws[e, dk] = t

            # ~4us of garbage matmuls while the HBM fill runs: trips the PE
            # HAM activity monitor to 8/8 (2.4 GHz) so the real stream starts
            # warm instead of paying ~2x on its first ~3.4us.
            wmt = bpool.tile([P, 64], mybir.dt.bfloat16, tag="warm")
            nc.vector.memset(wmt[:], 0)
            wps = pspool.tile([P, 64], f32, name="ps_warm", tag="ps")
            for i in range(90):
                nc.tensor.matmul(wps[0:64, :], wmt[:], wmt[:], start=True, stop=True)

            # persistent accumulator: one wide tile, sliced per (em,tn).
            # Slice-level deps proved structurally neutral vs 16 separate
            # tiles, and 15 fewer pool slots shortens the exit-protocol
            # semaphore sweep.
            acc_big = apool.tile([P, EM * TN * NT], f32, tag="accbig")
            accs = {}

            omerged = {}

            def epilogue(e, em, ps, tn_list=None, cols=None, dma_eng=None):
                bias_col = bias[:, e * EM + em: e * EM + em + 1]
                lo, hi = cols if cols is not None else (0, NT)
                for tn in (tn_list if tn_list is not None else range(TN)):
                    if e == 0:
                        i = em * TN + tn
                        acc = acc_big[:, i * NT:(i + 1) * NT]
                        accs[em, tn] = acc
                        if k == 1:
                            o = opool.tile([P, hi - lo], odt,
                                           name=f"o_{em}_{tn}_{lo}", tag="o")
                            nc.scalar.activation(
                                o[:], ps[tn][:, lo:hi],
                                mybir.ActivationFunctionType.Relu,
                                bias=bias_col)
                            nc.sync.dma_start(
                                out=outT_ap[em * P:(em + 1) * P,
                                            tn * NT + lo:tn * NT + hi],
                                in_=o[:])
                        else:
                            nc.scalar.activation(
                                acc[:, lo:hi], ps[tn][:, lo:hi],
                                mybir.ActivationFunctionType.Relu,
                                bias=bias_col)
                    elif e == k - 1:
                        acc = accs[em, tn]
                        r = rpool.tile([P, hi - lo], f32,
                                       name=f"r_{e}_{em}_{tn}_{lo}", tag="r")
                        nc.scalar.activation(
                            r[:], ps[tn][:, lo:hi],
                            mybir.ActivationFunctionType.Relu, bias=bias_col)
                        if cols is None and dma_eng is None:
                            # merged per-em output tile: one store per em
                            # (fewer 595ns DMA issues on the sync queue)
                            if em not in omerged:
                                omerged[em] = opool.tile(
                                    [P, TN * NT], odt, name=f"o_{em}", tag="o")
                            o = omerged[em]
                            nc.vector.tensor_add(
                                o[:, tn * NT:(tn + 1) * NT],
                                acc[:, lo:hi], r[:])
                            if tn == TN - 1:
                                nc.sync.dma_start(
                                    out=outT_ap[em * P:(em + 1) * P, :],
                                    in_=o[:])
                        else:
                            o = opool.tile([P, hi - lo], odt,
                                           name=f"o_{em}_{tn}_{lo}", tag="o")
                            nc.vector.tensor_add(o[:], acc[:, lo:hi], r[:])
                            (dma_eng or nc.sync).dma_start(
                                out=outT_ap[em * P:(em + 1) * P,
                                            tn * NT + lo:tn * NT + hi],
                                in_=o[:])
                    else:
                        acc = accs[em, tn]
                        r = rpool.tile([P, hi - lo], f32,
                                       name=f"r_{e}_{em}_{tn}_{lo}", tag="r")
                        nc.scalar.activation(
                            r[:], ps[tn][:, lo:hi],
                            mybir.ActivationFunctionType.Relu, bias=bias_col)
                        nc.vector.tensor_add(acc[:, lo:hi], acc[:, lo:hi],
                                             r[:])

            GW = 8 // TN  # em-groups per sweep (TN*GW psum banks in flight)
            for e in range(k):
                if e == 0:
                    # dk-major over GW concurrent groups: every arriving x/W
                    # strip immediately feeds TN*GW matmuls, so the PE never
                    # stalls on the HBM fill at kernel start.
                    for half in range(EM // GW):
                        groups = range(GW * half, GW * half + GW)
                        ps = {
                            g: [pspool.tile([P, NT], f32,
                                            name=f"ps_{e}_{g}_{tn}", tag="ps")
                                for tn in range(TN)]
                            for g in groups
                        }
                        for dk in range(DK):
                            for g in groups:
                                lhsT = ws[e, dk][:, g * P:(g + 1) * P]
                                for tn in range(TN):
                                    nc.tensor.matmul(
                                        ps[g][tn][:], lhsT,
                                        xs[dk][:, tn * NT:(tn + 1) * NT],
                                        start=(dk == 0), stop=(dk == DK - 1))
                        for g in groups:
                            epilogue(e, g, ps[g])
                else:
                    # data resident by now: plain em-major streaming
                    for em in range(EM):
                        ps = [
                            pspool.tile([P, NT], f32,
                                        name=f"ps_{e}_{em}_{tn}", tag="ps")
                            for tn in range(TN)
                        ]
                        if em == EM - 1 and e == k - 1 and _FINE_TAIL:
                            # last tile: tn=0 whole, tn=1 in two 256-col
                            # pieces, each fully finished (matmuls +
                            # relu/add/store) before the next. Stores fan out
                            # across idle engine queues so the final store's
                            # 595ns issue doesn't queue behind earlier ones.
                            for tn in range(TN):
                                if tn < TN - 1:
                                    pieces = [(0, NT, nc.gpsimd)]
                                else:
                                    pieces = [(0, NT // 2, nc.sync),
                                              (NT // 2, NT, nc.scalar)]
                                for (lo, hi, eng) in pieces:
                                    for dk in range(DK):
                                        nc.tensor.matmul(
                                            ps[tn][:, lo:hi],
                                            ws[e, dk][:, em * P:(em + 1) * P],
                                            xs[dk][:, tn * NT + lo:
                                                    tn * NT + hi],
                                            start=(dk == 0),
                                            stop=(dk == DK - 1))
                                    epilogue(e, em, ps, tn_list=[tn],
                                             cols=(lo, hi), dma_eng=eng)
                            continue
                        if em == EM - 1:
                            # tail: finish tile tn=0 completely first so its
                            # relu/add/store chain overlaps tn=1's matmuls
                            for tn in range(TN):
                                for dk in range(DK):
                                    nc.tensor.matmul(
                                        ps[tn][:],
                                        ws[e, dk][:, em * P:(em + 1) * P],
                                        xs[dk][:, tn * NT:(tn + 1) * NT],
                                        start=(dk == 0), stop=(dk == DK - 1))
                        else:
                            for dk in range(DK):
                                lhsT = ws[e, dk][:, em * P:(em + 1) * P]
                                for tn in range(TN):
                                    nc.tensor.matmul(
                                        ps[tn][:], lhsT,
                                        xs[dk][:, tn * NT:(tn + 1) * NT],
                                        start=(dk == 0), stop=(dk == DK - 1))
                        epilogue(e, em, ps)

            # post-stream keep-alive: the exit protocol's ~255-semaphore
            # reset sweep runs at whatever clock the activity monitor last
            # settled on; idle-gating to 4/8 doubles its length. Burn tiny
            # garbage matmuls through the epilogue-drain window to hold 8/8.
            if _TAIL_WARM:
                gps = [pspool.tile([P, 64], f32, name=f"gps{j}", tag="ps")
                       for j in range(2)]
                for i in range(_TAIL_WARM):
                    nc.tensor.matmul(gps[i % 2][0:64, :], wmt[:], wmt[:],
                                     start=True, stop=True)

    nc.compile()
    return nc


def _get_nc(k: int, dtype: str):
    key = (k, dtype)
    if key not in _nc_cache:
        _nc_cache[key] = _build(k, dtype)
    return _nc_cache[key]


def _prep_in_maps(x, logits, Ws, bs, k, dtype):
    x = np.asarray(x, dtype=np.float32)
    logits = np.asarray(logits, dtype=np.float32)
    Ws = np.asarray(Ws, dtype=np.float32)
    bs = np.asarray(bs, dtype=np.float32)

    # top-k by logits, descending, ties -> lower index (matches jax.lax.top_k)
    ids = np.argsort(-logits, kind="stable")[:k]

    npdt = _npdt(dtype)
    Wd = np.ascontiguousarray(Ws[ids].astype(npdt))              # [k, D, D]
    bT = np.ascontiguousarray(
        bs[ids].reshape(k, EM, P).transpose(2, 0, 1).reshape(P, k * EM)
    ).astype(np.float32)                                         # [P, k*EM]
    xT = x.astype(npdt).T                                        # [D, B] view

    in_maps = []
    for c in range(NCORES):
        in_maps.append({
            "xT": np.ascontiguousarray(xT[:, c * TPC:(c + 1) * TPC]),
            "w": Wd,
            "bT": bT,
        })
    return in_maps


def _gather(results):
    out = np.empty((B, D), dtype=np.float32)
    for c in range(NCORES):
        out[c * TPC:(c + 1) * TPC, :] = \
            np.asarray(results[c]["outT"]).astype(np.float32).T
    return out


def kernel(x, logits, Ws, bs, num_on_samples):
    k = int(num_on_samples)
    in_maps = _prep_in_maps(x, logits, Ws, bs, k, _DTYPE)
    nc = _get_nc(k, _DTYPE)
    res = run_bass_kernel_spmd(nc, in_maps, list(range(NCORES)))
    return _gather(res.results)


def run_traced(x, logits, Ws, bs, num_on_samples, dtype=None, **spmd_kwargs):
    """Dev helper: same as kernel() but returns (output, BassKernelResults)."""
    k = int(num_on_samples)
    dtype = dtype or _DTYPE
    in_maps = _prep_in_maps(x, logits, Ws, bs, k, dtype)
    nc = _get_nc(k, dtype)
    res = run_bass_kernel_spmd(nc, in_maps, list(range(NCORES)), **spmd_kwargs)
    return _gather(res.results), res



# revision 14
# speedup vs baseline: 1.0099x; 1.0041x over previous
"""MoE top-k routing kernel for Trainium2 (nn_MixedOp: top-2 of 8 Dense(1024->1024)+relu, summed).

Strategy:
  - Host: top-k selection over the 8 logits (tiny), slice the k selected expert
    weights/biases, transpose x so the contraction dim (D) is the SBUF
    partition dim (cast to the internal compute dtype).
  - Device: data-parallel shard of the 8192-token batch across 8 NeuronCores
    (1024 tokens/core), no collectives. Each core computes
        outT[:, t] = sum_e relu(W_e^T @ xT[:, t] + b_e)
    with PE matmuls (fp32 PSUM accumulate), relu+bias fused on the scalar
    engine, expert-sum on the vector engine. Expert-outer loop so expert e+1
    weights stream from HBM while expert e computes; the first expert runs
    dk-major over 4 concurrent PSUM groups so the PE never waits on the HBM
    fill; garbage warmup matmuls trip the PE clock gate to 2.4 GHz during the
    fill. x rides sync's HWDGE queue, W rides scalar's, in consumption order
    (each dma_start costs ~0.65us of sequencer issue time, and completion
    fires per whole transfer, so queue order = arrival order).
  - Host: transpose per-core outputs back and concatenate.

Measured (8 cores, bf16): 72.8-75us HW exec (best 72,842 ns), max-rel-err
~2.3e-3, resid_var ~4e-6 vs the fp32 reference. PE roofline ~55us; the rest
is the measured framework floor (~7us BSP preamble, ~4us HBM gating latency,
~6us exit protocol) — all verified invariant to kernel structure.
"""

import os
import sys
from contextlib import ExitStack

if "/opt/trn_rl_repo" not in sys.path:
    sys.path.insert(0, "/opt/trn_rl_repo")

import numpy as np
import ml_dtypes

import concourse.tile as tile
import concourse.bacc as bacc
import concourse.mybir as mybir
from concourse.bass_utils import run_bass_kernel_spmd

# bass_utils imports antenv.axon_hooks when tracing is requested (e.g. via a
# BASS_TRACE env var); the module is absent on some agent images — stub it so
# that path degrades to an untraced run instead of an ImportError.
try:
    import antenv.axon_hooks  # noqa: F401
except ImportError:
    import types as _types
    _m = _types.ModuleType("antenv.axon_hooks")
    _m.get_axon_ntff_profile_hook = lambda: None
    _m.set_axon_ntff_profile_hook = lambda h: None
    sys.modules["antenv.axon_hooks"] = _m

NCORES = 8
B = 8192
D = 1024
TPC = B // NCORES      # tokens per core
P = 128                # SBUF partitions
NT = 512               # matmul moving free-dim tile (one fp32 PSUM bank)
DK = D // P            # contraction tiles (8)
EM = D // P            # output-dim tiles (8)
TN = TPC // NT         # token tiles per core (2)

# internal compute dtype: "bf16" | "f32r" (fp32 data, full-rate reduced-precision
# PE mode) | "f32" (native fp32, 4x slower PE)
_DTYPE = os.environ.get("MOE_DTYPE", "bf16")
# of garbage matmuls appended after the real stream. Measured: the exit
# protocol's semaphore sweep paces at ~115ns/reset regardless of the HAM
# clock state, so keeping the clock up through the exit buys nothing.
_TAIL_WARM = int(os.environ.get("MOE_TAIL_WARM", "0"))
# store outputs as bf16 (half the store traffic; adds <=2^-9 relative error)
_BF16_OUT = os.environ.get("MOE_BF16_OUT", "1") == "1"
# split the last em-block of the last expert into 256-token chunks so the
# final relu+add+store chain after the very last matmul is short
_FINE_TAIL = os.environ.get("MOE_FINE_TAIL", "1") == "1"
# warmup garbage matmuls at kernel start. Each costs ~53ns of tensor-queue
# issue time; 90 of them occupied the queue until 12.3us while the first
# x/W strips were ready at ~9.3us. Size so warmup ends as the data lands.
_WARMUP = int(os.environ.get("MOE_WARMUP", "32"))

_nc_cache = {}


def _mdt(dtype: str):
    return {
        "bf16": mybir.dt.bfloat16,
        "f32r": mybir.dt.float32r,
        "f32": mybir.dt.float32,
    }[dtype]


def _npdt(dtype: str):
    return ml_dtypes.bfloat16 if dtype == "bf16" else np.float32


def _build(k: int, dtype: str):
    mdt = _mdt(dtype)
    f32 = mybir.dt.float32
    odt = mybir.dt.bfloat16 if _BF16_OUT else f32
    nc = bacc.Bacc("TRN2", debug=False, target_bir_lowering=False, num_devices=NCORES)
    xT_ap = nc.dram_tensor("xT", [D, TPC], mdt, kind="ExternalInput").ap()
    w_ap = nc.dram_tensor("w", [k, D, D], mdt, kind="ExternalInput").ap()
    bT_ap = nc.dram_tensor("bT", [P, k * EM], f32, kind="ExternalInput").ap()
    outT_ap = nc.dram_tensor("outT", [D, TPC], odt, kind="ExternalOutput").ap()

    with tile.TileContext(nc) as tc:
        with ExitStack() as ctx:
            xpool = ctx.enter_context(tc.tile_pool(name="x", bufs=1))
            wpool = ctx.enter_context(tc.tile_pool(name="w", bufs=1))
            bpool = ctx.enter_context(tc.tile_pool(name="b", bufs=1))
            pspool = ctx.enter_context(tc.tile_pool(name="ps", bufs=8, space="PSUM"))
            rpool = ctx.enter_context(tc.tile_pool(name="r", bufs=4))
            opool = ctx.enter_context(tc.tile_pool(name="o", bufs=4))
            apool = ctx.enter_context(tc.tile_pool(name="acc", bufs=1))

            # Queue discipline: HWDGE queues are per-engine FIFOs and a DMA's
            # completion semaphore fires only when the whole transfer is done,
            # so what shares a queue (and when) controls when the PE's gating
            # tiles land. x (+bias, +outputs later) ride sync's queue; W strips
            # ride scalar's queue in exact consumption order (expert 0 first).
            # wide tiles with per-strip DMAs into slices: slice-level dep
            # tracking keeps per-strip gating while using 1 pool slot each
            x_big = xpool.tile([P, DK * TPC], mdt, tag="xbig")
            xs = []
            for dk in range(DK):
                t = x_big[:, dk * TPC:(dk + 1) * TPC]
                nc.sync.dma_start(out=t, in_=xT_ap[dk * P:(dk + 1) * P, :])
                xs.append(t)

            # bias is tiny and first needed ~20us in; keep it off the head of
            # the x queue
            bias = bpool.tile([P, k * EM], f32, tag="bias")
            nc.sync.dma_start(out=bias[:], in_=bT_ap[:])

            ws = {}
            for e in range(k):
                w_big = wpool.tile([P, DK * D], mdt, name=f"w_big_{e}",
                                   tag=f"wbig{e}")
                for dk in range(DK):
                    t = w_big[:, dk * D:(dk + 1) * D]
                    nc.scalar.dma_start(out=t, in_=w_ap[e, dk * P:(dk + 1) * P, :])
                    ws[e, dk] = t

            # ~4us of garbage matmuls while the HBM fill runs: trips the PE
            # HAM activity monitor to 8/8 (2.4 GHz) so the real stream starts
            # warm instead of paying ~2x on its first ~3.4us.
            wmt = bpool.tile([P, 64], mybir.dt.bfloat16, tag="warm")
            nc.vector.memset(wmt[:], 0)
            wps = pspool.tile([P, 64], f32, name="ps_warm", tag="ps")
            for i in range(_WARMUP):
                nc.tensor.matmul(wps[0:64, :], wmt[:], wmt[:], start=True, stop=True)

            # persistent accumulator: one wide tile, sliced per (em,tn).
            # Slice-level deps proved structurally neutral vs 16 separate
            # tiles, and 15 fewer pool slots shortens the exit-protocol
            # semaphore sweep.
            acc_big = apool.tile([P, EM * TN * NT], f32, tag="accbig")
            accs = {}

            omerged = {}

            def epilogue(e, em, ps, tn_list=None, cols=None, dma_eng=None):
                bias_col = bias[:, e * EM + em: e * EM + em + 1]
                lo, hi = cols if cols is not None else (0, NT)
                for tn in (tn_list if tn_list is not None else range(TN)):
                    if e == 0:
                        i = em * TN + tn
                        acc = acc_big[:, i * NT:(i + 1) * NT]
                        accs[em, tn] = acc
                        if k == 1:
                            o = opool.tile([P, hi - lo], odt,
                                           name=f"o_{em}_{tn}_{lo}", tag="o")
                            nc.scalar.activation(
                                o[:], ps[tn][:, lo:hi],
                                mybir.ActivationFunctionType.Relu,
                                bias=bias_col)
                            nc.sync.dma_start(
                                out=outT_ap[em * P:(em + 1) * P,
                                            tn * NT + lo:tn * NT + hi],
                                in_=o[:])
                        else:
                            nc.scalar.activation(
                                acc[:, lo:hi], ps[tn][:, lo:hi],
                                mybir.ActivationFunctionType.Relu,
                                bias=bias_col)
                    elif e == k - 1:
                        acc = accs[em, tn]
                        r = rpool.tile([P, hi - lo], f32,
                                       name=f"r_{e}_{em}_{tn}_{lo}", tag="r")
                        nc.scalar.activation(
                            r[:], ps[tn][:, lo:hi],
                            mybir.ActivationFunctionType.Relu, bias=bias_col)
                        if cols is None and dma_eng is None:
                            # merged per-em output tile: one store per em
                            # (fewer 595ns DMA issues on the sync queue)
                            if em not in omerged:
                                omerged[em] = opool.tile(
                                    [P, TN * NT], odt, name=f"o_{em}", tag="o")
                            o = omerged[em]
                            nc.vector.tensor_add(
                                o[:, tn * NT:(tn + 1) * NT],
                                acc[:, lo:hi], r[:])
                            if tn == TN - 1:
                                nc.sync.dma_start(
                                    out=outT_ap[em * P:(em + 1) * P, :],
                                    in_=o[:])
                        else:
                            o = opool.tile([P, hi - lo], odt,
                                           name=f"o_{em}_{tn}_{lo}", tag="o")
                            nc.vector.tensor_add(o[:], acc[:, lo:hi], r[:])
                            (dma_eng or nc.sync).dma_start(
                                out=outT_ap[em * P:(em + 1) * P,
                                            tn * NT + lo:tn * NT + hi],
                                in_=o[:])
                    else:
                        acc = accs[em, tn]
                        r = rpool.tile([P, hi - lo], f32,
                                       name=f"r_{e}_{em}_{tn}_{lo}", tag="r")
                        nc.scalar.activation(
                            r[:], ps[tn][:, lo:hi],
                            mybir.ActivationFunctionType.Relu, bias=bias_col)
                        nc.vector.tensor_add(acc[:, lo:hi], acc[:, lo:hi],
                                             r[:])

            GW = 8 // TN  # em-groups per sweep (TN*GW psum banks in flight)
            for e in range(k):
                if e == 0:
                    # dk-major over GW concurrent groups: every arriving x/W
                    # strip immediately feeds TN*GW matmuls, so the PE never
                    # stalls on the HBM fill at kernel start.
                    for half in range(EM // GW):
                        groups = range(GW * half, GW * half + GW)
                        ps = {
                            g: [pspool.tile([P, NT], f32,
                                            name=f"ps_{e}_{g}_{tn}", tag="ps")
                                for tn in range(TN)]
                            for g in groups
                        }
                        for dk in range(DK):
                            for g in groups:
                                lhsT = ws[e, dk][:, g * P:(g + 1) * P]
                                for tn in range(TN):
                                    nc.tensor.matmul(
                                        ps[g][tn][:], lhsT,
                                        xs[dk][:, tn * NT:(tn + 1) * NT],
                                        start=(dk == 0), stop=(dk == DK - 1))
                        for g in groups:
                            epilogue(e, g, ps[g])
                else:
                    # data resident by now: plain em-major streaming
                    for em in range(EM):
                        if em == EM - 1 and e == k - 1 and _FINE_TAIL:
                            # last tile: tn=0 whole, tn=1 in two 256-col
                            # pieces. Each piece gets its OWN psum tile (a
                            # shared tile's start-flag would serialize piece
                            # N+1's matmuls behind piece N's relu) and its
                            # own store queue, so the final store's 595ns
                            # issue doesn't queue behind earlier ones.
                            bias_col = bias[:, e * EM + em: e * EM + em + 1]
                            pieces = [(0, 0, NT, nc.gpsimd),
                                      (1, 0, NT // 2, nc.sync),
                                      (1, NT // 2, NT, nc.scalar)]
                            for (tn, lo, hi, eng) in pieces:
                                pst = pspool.tile([P, hi - lo], f32,
                                                  name=f"psf_{tn}_{lo}",
                                                  tag="ps")
                                for dk in range(DK):
                                    nc.tensor.matmul(
                                        pst[:],
                                        ws[e, dk][:, em * P:(em + 1) * P],
                                        xs[dk][:, tn * NT + lo:tn * NT + hi],
                                        start=(dk == 0), stop=(dk == DK - 1))
                                r = rpool.tile([P, hi - lo], f32,
                                               name=f"rf_{tn}_{lo}", tag="r")
                                nc.scalar.activation(
                                    r[:], pst[:],
                                    mybir.ActivationFunctionType.Relu,
                                    bias=bias_col)
                                o = opool.tile([P, hi - lo], odt,
                                               name=f"of_{tn}_{lo}", tag="o")
                                nc.vector.tensor_add(
                                    o[:], accs[em, tn][:, lo:hi], r[:])
                                eng.dma_start(
                                    out=outT_ap[em * P:(em + 1) * P,
                                                tn * NT + lo:tn * NT + hi],
                                    in_=o[:])
                            continue
                        ps = [
                            pspool.tile([P, NT], f32,
                                        name=f"ps_{e}_{em}_{tn}", tag="ps")
                            for tn in range(TN)
                        ]
                        if em == EM - 1:
                            # tail: finish tile tn=0 completely first so its
                            # relu/add/store chain overlaps tn=1's matmuls
                            for tn in range(TN):
                                for dk in range(DK):
                                    nc.tensor.matmul(
                                        ps[tn][:],
                                        ws[e, dk][:, em * P:(em + 1) * P],
                                        xs[dk][:, tn * NT:(tn + 1) * NT],
                                        start=(dk == 0), stop=(dk == DK - 1))
                        else:
                            for dk in range(DK):
                                lhsT = ws[e, dk][:, em * P:(em + 1) * P]
                                for tn in range(TN):
                                    nc.tensor.matmul(
                                        ps[tn][:], lhsT,
                                        xs[dk][:, tn * NT:(tn + 1) * NT],
                                        start=(dk == 0), stop=(dk == DK - 1))
                        epilogue(e, em, ps)

            # post-stream keep-alive: the exit protocol's ~255-semaphore
            # reset sweep runs at whatever clock the activity monitor last
            # settled on; idle-gating to 4/8 doubles its length. Burn tiny
            # garbage matmuls through the epilogue-drain window to hold 8/8.
            if _TAIL_WARM:
                gps = [pspool.tile([P, 64], f32, name=f"gps{j}", tag="ps")
                       for j in range(2)]
                for i in range(_TAIL_WARM):
                    nc.tensor.matmul(gps[i % 2][0:64, :], wmt[:], wmt[:],
                                     start=True, stop=True)

    nc.compile()
    return nc


def _get_nc(k: int, dtype: str):
    key = (k, dtype)
    if key not in _nc_cache:
        _nc_cache[key] = _build(k, dtype)
    return _nc_cache[key]


def _prep_in_maps(x, logits, Ws, bs, k, dtype):
    x = np.asarray(x, dtype=np.float32)
    logits = np.asarray(logits, dtype=np.float32)
    Ws = np.asarray(Ws, dtype=np.float32)
    bs = np.asarray(bs, dtype=np.float32)

    # top-k by logits, descending, ties -> lower index (matches jax.lax.top_k)
    ids = np.argsort(-logits, kind="stable")[:k]

    npdt = _npdt(dtype)
    Wd = np.ascontiguousarray(Ws[ids].astype(npdt))              # [k, D, D]
    bT = np.ascontiguousarray(
        bs[ids].reshape(k, EM, P).transpose(2, 0, 1).reshape(P, k * EM)
    ).astype(np.float32)                                         # [P, k*EM]
    xT = x.astype(npdt).T                                        # [D, B] view

    in_maps = []
    for c in range(NCORES):
        in_maps.append({
            "xT": np.ascontiguousarray(xT[:, c * TPC:(c + 1) * TPC]),
            "w": Wd,
            "bT": bT,
        })
    return in_maps


def _gather(results):
    out = np.empty((B, D), dtype=np.float32)
    for c in range(NCORES):
        out[c * TPC:(c + 1) * TPC, :] = \
            np.asarray(results[c]["outT"]).astype(np.float32).T
    return out


def kernel(x, logits, Ws, bs, num_on_samples):
    k = int(num_on_samples)
    in_maps = _prep_in_maps(x, logits, Ws, bs, k, _DTYPE)
    nc = _get_nc(k, _DTYPE)
    res = run_bass_kernel_spmd(nc, in_maps, list(range(NCORES)))
    return _gather(res.results)


def run_traced(x, logits, Ws, bs, num_on_samples, dtype=None, **spmd_kwargs):
    """Dev helper: same as kernel() but returns (output, BassKernelResults)."""
    k = int(num_on_samples)
    dtype = dtype or _DTYPE
    in_maps = _prep_in_maps(x, logits, Ws, bs, k, dtype)
    nc = _get_nc(k, dtype)
    res = run_bass_kernel_spmd(nc, in_maps, list(range(NCORES)), **spmd_kwargs)
    return _gather(res.results), res



# revision 21
# speedup vs baseline: 1.0618x; 1.0513x over previous
"""MoE top-k routing kernel for Trainium2 (nn_MixedOp: top-2 of 8 Dense(1024->1024)+relu, summed).

Strategy:
  - Host: top-k selection over the 8 logits (tiny), slice the k selected expert
    weights/biases, transpose x so the contraction dim (D) is the SBUF
    partition dim (cast to the internal compute dtype).
  - Device: data-parallel shard of the 8192-token batch across 8 NeuronCores
    (1024 tokens/core), no collectives. Each core computes
        outT[:, t] = sum_e relu(W_e^T @ xT[:, t] + b_e)
    with PE matmuls (fp32 PSUM accumulate), relu+bias fused on the scalar
    engine, expert-sum on the vector engine. Expert-outer loop so expert e+1
    weights stream from HBM while expert e computes; the first expert runs
    dk-major over 4 concurrent PSUM groups so the PE never waits on the HBM
    fill; garbage warmup matmuls trip the PE clock gate to 2.4 GHz during the
    fill. x rides sync's HWDGE queue, W rides scalar's, in consumption order
    (each dma_start costs ~0.65us of sequencer issue time, and completion
    fires per whole transfer, so queue order = arrival order).
  - Host: transpose per-core outputs back and concatenate.

Measured (8 cores, bf16): 72.8-75us HW exec (best 72,842 ns), max-rel-err
~2.3e-3, resid_var ~4e-6 vs the fp32 reference. PE roofline ~55us; the rest
is the measured framework floor (~7us BSP preamble, ~4us HBM gating latency,
~6us exit protocol) — all verified invariant to kernel structure.
"""

import os
import sys
from contextlib import ExitStack

if "/opt/trn_rl_repo" not in sys.path:
    sys.path.insert(0, "/opt/trn_rl_repo")

import numpy as np
import ml_dtypes

import concourse.tile as tile
import concourse.bacc as bacc
import concourse.mybir as mybir
from concourse.bass_utils import run_bass_kernel_spmd

# bass_utils imports antenv.axon_hooks when tracing is requested (e.g. via a
# BASS_TRACE env var); the module is absent on some agent images — stub it so
# that path degrades to an untraced run instead of an ImportError.
try:
    import antenv.axon_hooks  # noqa: F401
except ImportError:
    import types as _types
    _m = _types.ModuleType("antenv.axon_hooks")
    _m.get_axon_ntff_profile_hook = lambda: None
    _m.set_axon_ntff_profile_hook = lambda h: None
    sys.modules["antenv.axon_hooks"] = _m

NCORES = 8
B = 8192
D = 1024
TPC = B // NCORES      # tokens per core
P = 128                # SBUF partitions
NT = 512               # matmul moving free-dim tile (one fp32 PSUM bank)
DK = D // P            # contraction tiles (8)
EM = D // P            # output-dim tiles (8)
TN = TPC // NT         # token tiles per core (2)

# internal compute dtype: "bf16" | "f32r" (fp32 data, full-rate reduced-precision
# PE mode) | "f32" (native fp32, 4x slower PE)
_DTYPE = os.environ.get("MOE_DTYPE", "bf16")
# of garbage matmuls appended after the real stream. Measured: the exit
# protocol's semaphore sweep paces at ~115ns/reset regardless of the HAM
# clock state, so keeping the clock up through the exit buys nothing.
_TAIL_WARM = int(os.environ.get("MOE_TAIL_WARM", "0"))
# store outputs as bf16 (half the store traffic; adds <=2^-9 relative error)
_BF16_OUT = os.environ.get("MOE_BF16_OUT", "1") == "1"
# split the last em-block of the last expert into 256-token chunks so the
# final relu+add+store chain after the very last matmul is short
_FINE_TAIL = os.environ.get("MOE_FINE_TAIL", "1") == "1"
# warmup garbage matmuls at kernel start. Each costs ~53ns of tensor-queue
# issue time; 90 of them occupied the queue until 12.3us while the first
# x/W strips were ready at ~9.3us. Size so warmup ends as the data lands.
_WARMUP = int(os.environ.get("MOE_WARMUP", "32"))
# compute expert 0's first two K-strips (256 of 1024 contraction) as ONE fp8
# DoubleRow matmul instead of two bf16 matmuls: -1/16 of PE time (~3.4us over
# 8 cores). fp8 e4m3 quantization of that 1/8 of expert 0's contraction
# raises max_rel from 3.8e-3 to 1.53e-2 (gate is 2e-2; verified vs fp64).
_FP8_PAIRS = int(os.environ.get("MOE_FP8_PAIRS", "1"))

_nc_cache = {}


def _mdt(dtype: str):
    return {
        "bf16": mybir.dt.bfloat16,
        "f32r": mybir.dt.float32r,
        "f32": mybir.dt.float32,
    }[dtype]


def _npdt(dtype: str):
    return ml_dtypes.bfloat16 if dtype == "bf16" else np.float32


def _build(k: int, dtype: str):
    mdt = _mdt(dtype)
    f32 = mybir.dt.float32
    f8 = mybir.dt.float8e4
    odt = mybir.dt.bfloat16 if _BF16_OUT else f32
    fp8_pair = bool(_FP8_PAIRS) and dtype == "bf16"
    nc = bacc.Bacc("TRN2", debug=False, target_bir_lowering=False, num_devices=NCORES)
    xT_ap = nc.dram_tensor("xT", [D, TPC], mdt, kind="ExternalInput").ap()
    w_ap = nc.dram_tensor("w", [k, D, D], mdt, kind="ExternalInput").ap()
    bT_ap = nc.dram_tensor("bT", [P, k * EM], f32, kind="ExternalInput").ap()
    outT_ap = nc.dram_tensor("outT", [D, TPC], odt, kind="ExternalOutput").ap()
    if fp8_pair:
        xf8_ap = nc.dram_tensor("xf8", [P, 2, TPC], f8, kind="ExternalInput").ap()
        wf8_ap = nc.dram_tensor("wf8", [P, 2, D], f8, kind="ExternalInput").ap()

    with tile.TileContext(nc) as tc:
        with ExitStack() as ctx:
            xpool = ctx.enter_context(tc.tile_pool(name="x", bufs=1))
            wpool = ctx.enter_context(tc.tile_pool(name="w", bufs=1))
            bpool = ctx.enter_context(tc.tile_pool(name="b", bufs=1))
            pspool = ctx.enter_context(tc.tile_pool(name="ps", bufs=8, space="PSUM"))
            rpool = ctx.enter_context(tc.tile_pool(name="r", bufs=4))
            opool = ctx.enter_context(tc.tile_pool(name="o", bufs=4))
            apool = ctx.enter_context(tc.tile_pool(name="acc", bufs=1))

            # Queue discipline: HWDGE queues are per-engine FIFOs and a DMA's
            # completion semaphore fires only when the whole transfer is done,
            # so what shares a queue (and when) controls when the PE's gating
            # tiles land. x (+bias, +outputs later) ride sync's queue; W strips
            # ride scalar's queue in exact consumption order (expert 0 first).
            # wide tiles with per-strip DMAs into slices: slice-level dep
            # tracking keeps per-strip gating while using 1 pool slot each.
            # With the fp8 pair, the pair's small fp8 tiles go FIRST on each
            # queue (first compute needs only them), and the bf16 dk0/dk1
            # x strips (only needed by expert 1, ~25us later) go last.
            if fp8_pair:
                xf8_t = xpool.tile([P, 2, TPC], f8, name="xf8", tag="xf8")
                nc.sync.dma_start(out=xf8_t[:], in_=xf8_ap[:])
                wf8_t = wpool.tile([P, 2, D], f8, name="wf8", tag="wf8")
                nc.scalar.dma_start(out=wf8_t[:], in_=wf8_ap[:])
                x_order = list(range(2, DK)) + [0, 1]
            else:
                x_order = list(range(DK))
            x_big = xpool.tile([P, DK * TPC], mdt, tag="xbig")
            xs = [x_big[:, dk * TPC:(dk + 1) * TPC] for dk in range(DK)]
            for dk in x_order:
                nc.sync.dma_start(out=xs[dk], in_=xT_ap[dk * P:(dk + 1) * P, :])

            # bias is tiny and first needed ~20us in; keep it off the head of
            # the x queue
            bias = bpool.tile([P, k * EM], f32, tag="bias")
            nc.sync.dma_start(out=bias[:], in_=bT_ap[:])

            ws = {}
            for e in range(k):
                w_big = wpool.tile([P, DK * D], mdt, name=f"w_big_{e}",
                                   tag=f"wbig{e}")
                for dk in range(DK):
                    if fp8_pair and e == 0 and dk < 2:
                        continue  # covered by the fp8 DoubleRow pair
                    t = w_big[:, dk * D:(dk + 1) * D]
                    nc.scalar.dma_start(out=t, in_=w_ap[e, dk * P:(dk + 1) * P, :])
                    ws[e, dk] = t

            # ~4us of garbage matmuls while the HBM fill runs: trips the PE
            # HAM activity monitor to 8/8 (2.4 GHz) so the real stream starts
            # warm instead of paying ~2x on its first ~3.4us.
            wmt = bpool.tile([P, 64], mybir.dt.bfloat16, tag="warm")
            nc.vector.memset(wmt[:], 0)
            wps = pspool.tile([P, 64], f32, name="ps_warm", tag="ps")
            for i in range(_WARMUP):
                nc.tensor.matmul(wps[0:64, :], wmt[:], wmt[:], start=True, stop=True)

            # persistent accumulator: one wide tile, sliced per (em,tn).
            # Slice-level deps proved structurally neutral vs 16 separate
            # tiles, and 15 fewer pool slots shortens the exit-protocol
            # semaphore sweep.
            acc_big = apool.tile([P, EM * TN * NT], f32, tag="accbig")
            accs = {}

            omerged = {}

            def epilogue(e, em, ps, tn_list=None, cols=None, dma_eng=None):
                bias_col = bias[:, e * EM + em: e * EM + em + 1]
                lo, hi = cols if cols is not None else (0, NT)
                for tn in (tn_list if tn_list is not None else range(TN)):
                    if e == 0:
                        i = em * TN + tn
                        acc = acc_big[:, i * NT:(i + 1) * NT]
                        accs[em, tn] = acc
                        if k == 1:
                            o = opool.tile([P, hi - lo], odt,
                                           name=f"o_{em}_{tn}_{lo}", tag="o")
                            nc.scalar.activation(
                                o[:], ps[tn][:, lo:hi],
                                mybir.ActivationFunctionType.Relu,
                                bias=bias_col)
                            nc.sync.dma_start(
                                out=outT_ap[em * P:(em + 1) * P,
                                            tn * NT + lo:tn * NT + hi],
                                in_=o[:])
                        else:
                            nc.scalar.activation(
                                acc[:, lo:hi], ps[tn][:, lo:hi],
                                mybir.ActivationFunctionType.Relu,
                                bias=bias_col)
                    elif e == k - 1:
                        acc = accs[em, tn]
                        r = rpool.tile([P, hi - lo], f32,
                                       name=f"r_{e}_{em}_{tn}_{lo}", tag="r")
                        nc.scalar.activation(
                            r[:], ps[tn][:, lo:hi],
                            mybir.ActivationFunctionType.Relu, bias=bias_col)
                        if cols is None and dma_eng is None:
                            # merged per-em output tile: one store per em
                            # (fewer 595ns DMA issues on the sync queue)
                            if em not in omerged:
                                omerged[em] = opool.tile(
                                    [P, TN * NT], odt, name=f"o_{em}", tag="o")
                            o = omerged[em]
                            nc.vector.tensor_add(
                                o[:, tn * NT:(tn + 1) * NT],
                                acc[:, lo:hi], r[:])
                            if tn == TN - 1:
                                nc.sync.dma_start(
                                    out=outT_ap[em * P:(em + 1) * P, :],
                                    in_=o[:])
                        else:
                            o = opool.tile([P, hi - lo], odt,
                                           name=f"o_{em}_{tn}_{lo}", tag="o")
                            nc.vector.tensor_add(o[:], acc[:, lo:hi], r[:])
                            (dma_eng or nc.sync).dma_start(
                                out=outT_ap[em * P:(em + 1) * P,
                                            tn * NT + lo:tn * NT + hi],
                                in_=o[:])
                    else:
                        acc = accs[em, tn]
                        r = rpool.tile([P, hi - lo], f32,
                                       name=f"r_{e}_{em}_{tn}_{lo}", tag="r")
                        nc.scalar.activation(
                            r[:], ps[tn][:, lo:hi],
                            mybir.ActivationFunctionType.Relu, bias=bias_col)
                        nc.vector.tensor_add(acc[:, lo:hi], acc[:, lo:hi],
                                             r[:])

            GW = 8 // TN  # em-groups per sweep (TN*GW psum banks in flight)
            for e in range(k):
                if e == 0:
                    # dk-major over GW concurrent groups: every arriving x/W
                    # strip immediately feeds TN*GW matmuls, so the PE never
                    # stalls on the HBM fill at kernel start. The fp8
                    # DoubleRow pair (K-strips 0+1 in one matmul) leads each
                    # group: it only needs the small fp8 tiles at the head of
                    # both DMA queues.
                    dk0 = 2 if fp8_pair else 0
                    for half in range(EM // GW):
                        groups = range(GW * half, GW * half + GW)
                        ps = {
                            g: [pspool.tile([P, NT], f32,
                                            name=f"ps_{e}_{g}_{tn}", tag="ps")
                                for tn in range(TN)]
                            for g in groups
                        }
                        if fp8_pair:
                            for g in groups:
                                lhsT8 = wf8_t[:, :, g * P:(g + 1) * P]
                                for tn in range(TN):
                                    nc.tensor.matmul(
                                        ps[g][tn][:], lhsT8,
                                        xf8_t[:, :, tn * NT:(tn + 1) * NT],
                                        start=True, stop=False,
                                        perf_mode=mybir.MatmulPerfMode.DoubleRow)
                        for dk in range(dk0, DK):
                            for g in groups:
                                lhsT = ws[e, dk][:, g * P:(g + 1) * P]
                                for tn in range(TN):
                                    nc.tensor.matmul(
                                        ps[g][tn][:], lhsT,
                                        xs[dk][:, tn * NT:(tn + 1) * NT],
                                        start=(dk == dk0 and not fp8_pair),
                                        stop=(dk == DK - 1))
                        for g in groups:
                            epilogue(e, g, ps[g])
                else:
                    # data resident by now: plain em-major streaming
                    for em in range(EM):
                        if em == EM - 1 and e == k - 1 and _FINE_TAIL:
                            # last tile: tn=0 whole, tn=1 in two 256-col
                            # pieces. Each piece gets its OWN psum tile (a
                            # shared tile's start-flag would serialize piece
                            # N+1's matmuls behind piece N's relu), and each
                            # piece's relu/add/store chain runs on engines
                            # whose queues are clear at that moment, so the
                            # three chains overlap instead of serializing on
                            # scalar (relus), vector (adds) and one DMA
                            # queue (595ns per store issue).
                            bias_col = bias[:, e * EM + em: e * EM + em + 1]
                            # (tn, lo, hi, dma_eng); emission order = PE
                            # order, so each piece's relu starts right as
                            # its matmuls finish (gpsimd can't read PSUM,
                            # so relus stay on scalar, adds on vector)
                            pieces = [(1, 0, NT // 2, nc.gpsimd),
                                      (0, 0, NT, nc.sync),
                                      (1, NT // 2, NT, nc.scalar)]
                            for (tn, lo, hi, deng) in pieces:
                                pst = pspool.tile([P, hi - lo], f32,
                                                  name=f"psf_{tn}_{lo}",
                                                  tag="ps")
                                for dk in range(DK):
                                    nc.tensor.matmul(
                                        pst[:],
                                        ws[e, dk][:, em * P:(em + 1) * P],
                                        xs[dk][:, tn * NT + lo:tn * NT + hi],
                                        start=(dk == 0), stop=(dk == DK - 1))
                                r = rpool.tile([P, hi - lo], f32,
                                               name=f"rf_{tn}_{lo}", tag="r")
                                nc.scalar.activation(
                                    r[:], pst[:],
                                    mybir.ActivationFunctionType.Relu,
                                    bias=bias_col)
                                o = opool.tile([P, hi - lo], odt,
                                               name=f"of_{tn}_{lo}", tag="o")
                                nc.vector.tensor_add(
                                    o[:], accs[em, tn][:, lo:hi], r[:])
                                deng.dma_start(
                                    out=outT_ap[em * P:(em + 1) * P,
                                                tn * NT + lo:tn * NT + hi],
                                    in_=o[:])
                            continue
                        ps = [
                            pspool.tile([P, NT], f32,
                                        name=f"ps_{e}_{em}_{tn}", tag="ps")
                            for tn in range(TN)
                        ]
                        if em == EM - 1:
                            # tail: finish tile tn=0 completely first so its
                            # relu/add/store chain overlaps tn=1's matmuls
                            for tn in range(TN):
                                for dk in range(DK):
                                    nc.tensor.matmul(
                                        ps[tn][:],
                                        ws[e, dk][:, em * P:(em + 1) * P],
                                        xs[dk][:, tn * NT:(tn + 1) * NT],
                                        start=(dk == 0), stop=(dk == DK - 1))
                        else:
                            for dk in range(DK):
                                lhsT = ws[e, dk][:, em * P:(em + 1) * P]
                                for tn in range(TN):
                                    nc.tensor.matmul(
                                        ps[tn][:], lhsT,
                                        xs[dk][:, tn * NT:(tn + 1) * NT],
                                        start=(dk == 0), stop=(dk == DK - 1))
                        epilogue(e, em, ps)

            # post-stream keep-alive: the exit protocol's ~255-semaphore
            # reset sweep runs at whatever clock the activity monitor last
            # settled on; idle-gating to 4/8 doubles its length. Burn tiny
            # garbage matmuls through the epilogue-drain window to hold 8/8.
            if _TAIL_WARM:
                gps = [pspool.tile([P, 64], f32, name=f"gps{j}", tag="ps")
                       for j in range(2)]
                for i in range(_TAIL_WARM):
                    nc.tensor.matmul(gps[i % 2][0:64, :], wmt[:], wmt[:],
                                     start=True, stop=True)

    nc.compile()
    return nc


def _get_nc(k: int, dtype: str):
    key = (k, dtype)
    if key not in _nc_cache:
        _nc_cache[key] = _build(k, dtype)
    return _nc_cache[key]


def _prep_in_maps(x, logits, Ws, bs, k, dtype):
    x = np.asarray(x, dtype=np.float32)
    logits = np.asarray(logits, dtype=np.float32)
    Ws = np.asarray(Ws, dtype=np.float32)
    bs = np.asarray(bs, dtype=np.float32)

    # top-k by logits, descending, ties -> lower index (matches jax.lax.top_k)
    ids = np.argsort(-logits, kind="stable")[:k]

    npdt = _npdt(dtype)
    Wd = np.ascontiguousarray(Ws[ids].astype(npdt))              # [k, D, D]
    bT = np.ascontiguousarray(
        bs[ids].reshape(k, EM, P).transpose(2, 0, 1).reshape(P, k * EM)
    ).astype(np.float32)                                         # [P, k*EM]
    xT = x.astype(npdt).T                                        # [D, B] view

    fp8_pair = bool(_FP8_PAIRS) and dtype == "bf16"
    if fp8_pair:
        f8 = ml_dtypes.float8_e4m3
        # expert 0 strips dk0,dk1: [P, 2, *] strip-interleaved fp8, quantized
        # from the original f32 data (not the bf16 copies)
        xT8 = x[:, 0:2 * P].astype(f8).T                         # [256, B]
        wq = Ws[ids[0]][0:2 * P, :].astype(f8)                   # [256, D]
        wf8 = np.ascontiguousarray(np.stack([wq[0:P], wq[P:2 * P]], axis=1))

    in_maps = []
    for c in range(NCORES):
        im = {
            "xT": np.ascontiguousarray(xT[:, c * TPC:(c + 1) * TPC]),
            "w": Wd,
            "bT": bT,
        }
        if fp8_pair:
            xc = xT8[:, c * TPC:(c + 1) * TPC]
            im["xf8"] = np.ascontiguousarray(
                np.stack([xc[0:P], xc[P:2 * P]], axis=1))        # [P, 2, TPC]
            im["wf8"] = wf8                                      # [P, 2, D]
        in_maps.append(im)
    return in_maps


def _gather(results):
    out = np.empty((B, D), dtype=np.float32)
    for c in range(NCORES):
        out[c * TPC:(c + 1) * TPC, :] = \
            np.asarray(results[c]["outT"]).astype(np.float32).T
    return out


def kernel(x, logits, Ws, bs, num_on_samples):
    k = int(num_on_samples)
    in_maps = _prep_in_maps(x, logits, Ws, bs, k, _DTYPE)
    nc = _get_nc(k, _DTYPE)
    res = run_bass_kernel_spmd(nc, in_maps, list(range(NCORES)))
    return _gather(res.results)


def run_traced(x, logits, Ws, bs, num_on_samples, dtype=None, **spmd_kwargs):
    """Dev helper: same as kernel() but returns (output, BassKernelResults)."""
    k = int(num_on_samples)
    dtype = dtype or _DTYPE
    in_maps = _prep_in_maps(x, logits, Ws, bs, k, dtype)
    nc = _get_nc(k, dtype)
    res = run_bass_kernel_spmd(nc, in_maps, list(range(NCORES)), **spmd_kwargs)
    return _gather(res.results), res

